# revision 58
# baseline (speedup 1.0000x reference)
"""Trainium2 Bass kernel for causal self-attention (RoPE + per-head RMSNorm).

Reference computation (B=2, T=2048, C=1024, H=16, D=64):
    q = rope(rmsnorm(x @ Wq.T)); k = rope(rmsnorm(x @ Wk.T)); v = x @ Wv.T
    out = softmax(causal(q k^T / sqrt(D))) v @ Wo.T

Sharding over 8 NeuronCores: core c -> batch b = c//4, head-group g = c%4
(4 heads = 256 features per group).  Feature-major ("transposed") on-chip
layout; scores computed as S^T[tk, tq]; softmax denominator via a ones
column appended to V (attn-V matmul M=65).

v2.2 structure:
  - packed host-side input layouts -> 8 big contiguous input DMAs.
  - scores for the two heads of a head-pair in one 2-bank PSUM tile
    [128,1024]; one exp activation covers both heads' tiles.
  - causal shrink: score/AV matmuls and exp skip fully-masked column
    ranges; only the diagonal 128x128 block is masked (DVE).
  - ALL reciprocals via exp(-ln(x)) on ACT (same activation table set as
    the attention exp -> zero ACT table reloads, no DVE casts).
  - engine roles: ACT = exp/ln/square only; DVE = elementwise+masks;
    GPSIMD = rope mul/add (slack-tolerant) + collectives + bounce DMA;
    PE = all matmuls; sync = loads/unbounce/output.
  - dense PE stream: proj(mt=1) woven into attention(hp=0), o_proj wave0
    woven into attention(hp=1), second-head AV passes woven into the next
    query block's score stream.
  - AllToAll split into 4 pieces (hp x query-block pairs); received slots
    selected dynamically via cc_rank (no junk slots, halved wo/o_proj).
  - last o_proj wave split into column halves so only the final quarter
    of the A2A + a small wave is exposed at the tail.
"""

import os
import sys

for _p in ("/opt/trn_rl_repo", "/root/.axon_site/_ro/trn_rl_repo"):
    if os.path.isdir(_p) and _p not in sys.path:
        sys.path.insert(0, _p)

import numpy as np
import ml_dtypes

import concourse.bass as bass
from concourse import bacc
import concourse.tile as tile
import concourse.mybir as mybir

# Pin activation-table selection to the one set that covers every ACT
# function this kernel uses (exp/ln/square/copy/identity).  The default
# greedy per-instruction selection alternates between the exp-only and
# ln-only sets, inserting a 1.3us ACT_TABLE_LOAD at every switch (60+
# loads).  Masking the other sets (positions preserved, so the emitted
# act_func_set_id still indexes act_info.json correctly) forces every
# activation onto the combined resident table -> one load total.
_COMBO_SET = "natural_log_exp_and_others"
_orig_get_tables = bacc.get_activation_tables


def _mono_tables(arch):
    t = _orig_get_tables(arch)
    if _COMBO_SET not in t:
        return t
    return {k: (v if k == _COMBO_SET else set()) for k, v in t.items()}


bacc.get_activation_tables = _mono_tables

BF16 = mybir.dt.bfloat16
F32 = mybir.dt.float32
AF = mybir.ActivationFunctionType

B, T, C, H, D = 2, 2048, 1024, 16, 64
N_CORES = 8
GH = 4  # heads per core
GF = GH * D  # features per core (256)
TB = 512  # token block (matmul N)
KT = C // 128  # 8 contraction k-tiles
EPS = float(np.finfo(np.float32).eps)
ROPE_BASE = 10000.0
ISD = 1.0 / np.sqrt(D)


def build_nc(t=T):
    ntb = t // TB  # query blocks (4)
    ntt = t // 128  # token 128-tiles (16)
    tsl = t // 4  # per-core o_proj token count (512)
    nst = tsl // ntb  # stripe width (128)
    th = t // 2

    nc = bacc.Bacc("TRN2", target_bir_lowering=False, debug=False, num_devices=N_CORES)

    # packed inputs (one contiguous DMA each)
    xlo = nc.dram_tensor("xlo", [128, KT * th], BF16, kind="ExternalInput")
    xhi = nc.dram_tensor("xhi", [128, KT * th], BF16, kind="ExternalInput")
    wqp = nc.dram_tensor("wqp", [128, KT * GF], BF16, kind="ExternalInput")
    wkp = nc.dram_tensor("wkp", [128, KT * GF], BF16, kind="ExternalInput")
    wvp = nc.dram_tensor("wvp", [128, KT * GF], BF16, kind="ExternalInput")
    wop = nc.dram_tensor("wop", [128, KT * C], BF16, kind="ExternalInput")
    cstab = nc.dram_tensor("cstab", [128, 4 * t], BF16, kind="ExternalInput")
    tabp = nc.dram_tensor("tabp", [128, 516], BF16, kind="ExternalInput")
    out = nc.dram_tensor("out", [C, tsl], F32, kind="ExternalOutput")

    with tile.TileContext(nc) as tc:
        with (
            nc.allow_low_precision(reason="bf16 compute by design"),
            tc.tile_pool(name="p_x", bufs=2) as p_x,
            tc.tile_pool(name="p_w", bufs=3) as p_w,
            tc.tile_pool(name="p_tab", bufs=1) as p_tab,
            tc.tile_pool(name="p_qk", bufs=2) as p_qk,
            tc.tile_pool(name="p_v", bufs=ntt) as p_v,
            tc.tile_pool(name="p_y", bufs=1) as p_y,
            tc.tile_pool(name="p_yg", bufs=KT) as p_yg,
            tc.tile_pool(name="p_pt", bufs=14) as p_pt,
            tc.tile_pool(name="p_tmp", bufs=3) as p_tmp,
            tc.tile_pool(name="p_oacc", bufs=KT) as p_oacc,
            tc.tile_pool(name="p_mm2", bufs=2, space="PSUM") as p_mm2,
            tc.tile_pool(name="p_po", bufs=2, space="PSUM") as p_po,
            tc.tile_pool(name="p_wk", bufs=2, space="PSUM") as p_wk,
            tc.tile_pool(name="p_dram", bufs=2, space="DRAM") as p_dram,
        ):
            # ---- input loads: big DMAs on the sync+scalar queues only.
            # The gpsimd queue is unusable for ~45us at kernel start (the
            # collectives-init op occupies it), so nothing startup-critical
            # goes there.
            tab_t = p_tab.tile([128, 516], BF16, tag="tab", name="tab_t")
            nc.sync.dma_start(tab_t[:], tabp[:])
            wq_t = p_w.tile([128, KT * GF], BF16, tag="wq", name="wq_t", bufs=1)
            nc.scalar.dma_start(wq_t[:], wqp[:])
            xlo_t = p_x.tile([128, KT * th], BF16, tag="xl", name="xlo_t", bufs=1)
            nc.sync.dma_start(xlo_t[:, 0 : 4 * th], xlo[:, 0 : 4 * th])
            nc.scalar.dma_start(xlo_t[:, 4 * th : 8 * th], xlo[:, 4 * th : 8 * th])
            wv_t = p_w.tile([128, KT * GF], BF16, tag="wv", name="wv_t", bufs=1)
            nc.scalar.dma_start(wv_t[:], wvp[:])
            wk_t = p_w.tile([128, KT * GF], BF16, tag="wk", name="wk_t", bufs=1)
            nc.scalar.dma_start(wk_t[:], wkp[:])
            xhi_t = p_x.tile([128, KT * th], BF16, tag="xh", name="xhi_t", bufs=1)
            nc.sync.dma_start(xhi_t[:, 0 : 4 * th], xhi[:, 0 : 4 * th])
            nc.sync.dma_start(xhi_t[:, 4 * th : 8 * th], xhi[:, 4 * th : 8 * th])
            cs_t = p_tab.tile([128, 4 * t], BF16, tag="cs", name="cs_t")
            nc.scalar.dma_start(cs_t[:, 0 : 2 * t], cstab[:, 0 : 2 * t])
            nc.sync.dma_start(cs_t[:, 2 * t : 4 * t], cstab[:, 2 * t : 4 * t])
            # wo is needed only ~150us in; the gpsimd queue (blocked early by
            # the collectives-init) is fine for it and keeps sync free.
            wo_t = p_w.tile([128, KT * C], BF16, tag="wo", name="wo_t", bufs=1)
            nc.gpsimd.dma_start(wo_t[:], wop[:])

            def xsl(ct, jb):  # x^T tile ct, token slice jb (global cols)
                lo, hi = jb.start, jb.stop
                if hi <= th:
                    return xlo_t[:, ct * th + lo : ct * th + hi]
                return xhi_t[:, ct * th + lo - th : ct * th + hi - th]

            def wsl(w, ct, mt=None):
                base = w[:, ct * GF : (ct + 1) * GF]
                if mt is None:
                    return base
                return w[:, ct * GF + mt * 128 : ct * GF + (mt + 1) * 128]

            pswap_sb = tab_t[:, 0:128]
            blk2_sb = tab_t[:, 128:130]
            mask2_sb = tab_t[:, 130:386]
            e2b_sb = tab_t[0:2, 388:516]
            cosq_sb = cs_t[:, 0 * t : 1 * t]
            sinq_sb = cs_t[:, 1 * t : 2 * t]
            cosk_sb = cs_t[:, 2 * t : 3 * t]
            sink_sb = cs_t[:, 3 * t : 4 * t]
            eps_sb = p_tab.tile([128, 1], F32, tag="eps")
            nc.vector.memset(eps_sb[:], EPS)

            # register with 4*(rank//4) for dynamic slot selection from A2A
            rb_reg = nc.sync.alloc_register()
            nc.sync.cc_rank_ld(rb_reg, replica_groups=[[0, 1, 2, 3, 4, 5, 6, 7]])
            rg_reg = nc.sync.alloc_register()
            nc.sync.reg_mod(rg_reg, rb_reg, 4)
            nc.sync.reg_div(rb_reg, rb_reg, 4)
            nc.sync.reg_mul(rb_reg, rb_reg, 4)
            rb4s = nc.sync.snap(rb_reg, min_val=0, max_val=4)
            rg4s = nc.sync.snap(rg_reg, min_val=0, max_val=3)

            qh_sb = [p_qk.tile([128, t], BF16, tag="qk0", name="qh0"),
                     p_qk.tile([128, t], BF16, tag="qk1", name="qh1")]
            kh_sb = [p_qk.tile([128, t], BF16, tag="qk0", name="kh0"),
                     p_qk.tile([128, t], BF16, tag="qk1", name="kh1")]
            v_sb = []
            y_sb = [p_y.tile([128, t], BF16, tag="y0", name="y0"),
                    p_y.tile([128, t], BF16, tag="y1", name="y1")]
            yg_sb = [None] * KT  # index m = 2*gi + hp
            for i in range(KT):
                yg_sb[i] = p_yg.tile([128, tsl], BF16, tag="yg", name=f"yg{i}")
            oacc_sb = []
            for co in range(KT):
                o_t = p_oacc.tile([128, tsl], BF16, tag="oacc", name=f"oac{co}")
                oacc_sb.append(o_t)

            # ---- proj block: 8 acc MMs + rmsnorm/rope chain ------------------
            def emit_proj(which, mt, j):
                jb = slice(j * TB, (j + 1) * TB)
                w_t = wq_t if which == "q" else wk_t
                cos_sb = cosq_sb if which == "q" else cosk_sb
                sin_sb = sinq_sb if which == "q" else sink_sb
                dst = (qh_sb if which == "q" else kh_sb)[mt]
                acc = p_wk.tile([128, TB], F32, tag="w", name="acc")
                for ct in range(KT):
                    nc.tensor.matmul(
                        acc[:],
                        wsl(w_t, ct, mt),
                        xsl(ct, jb),
                        start=(ct == 0),
                        stop=(ct == KT - 1),
                    )
                # rmsnorm deferred past rope: inv is constant within a head's
                # 64 rows, so it commutes with the half-swap rotation.
                pqb = p_tmp.tile([128, TB], BF16, tag="qn", name="pqb")
                nc.vector.tensor_copy(pqb[:], acc[:])
                sq = p_tmp.tile([128, TB], BF16, tag="sq", name="sq", bufs=2)
                nc.vector.tensor_mul(sq[:], pqb[:], pqb[:])
                # pw holds the [2,TB] sum-of-squares first, then (after ln
                # consumed it) the [128,TB] broadcast inv: one PSUM bank.
                pw = p_wk.tile([128, TB], F32, tag="w", name="pw")
                nc.tensor.matmul(pw[0:2, :], blk2_sb, sq[:], start=True, stop=True)
                # inv = 1/sqrt(ms+eps) = exp(-0.5*ln(ms+eps)): stays in the
                # exp/ln ACT table set (no table reloads).
                lnv = p_tmp.tile([2, TB], F32, tag="lnv", name="lnv", bufs=2)
                nc.scalar.activation(
                    lnv[:], pw[0:2, :], AF.Ln, scale=1.0 / D, bias=eps_sb[0:2, :]
                )
                invb = p_tmp.tile([2, TB], BF16, tag="invb", name="invb", bufs=2)
                nc.scalar.activation(invb[:], lnv[:], AF.Exp, scale=-0.5)
                nc.tensor.matmul(pw[:], e2b_sb, invb[:], start=True, stop=True)
                pqs = p_wk.tile([128, TB], F32, tag="w", name="pqs")
                nc.tensor.matmul(pqs[:], pswap_sb, pqb[:], start=True, stop=True)
                t1 = p_tmp.tile([128, TB], BF16, tag="t1", name="t1")
                nc.gpsimd.tensor_mul(t1[:], pqb[:], cos_sb[:, jb])
                t2 = p_tmp.tile([128, TB], BF16, tag="t2", name="t2")
                nc.vector.tensor_mul(t2[:], pqs[:], sin_sb[:, jb])
                rsum = p_tmp.tile([128, TB], BF16, tag="rs", name="rsum")
                nc.gpsimd.tensor_add(rsum[:], t1[:], t2[:])
                nc.vector.tensor_mul(dst[:, jb], rsum[:], pw[:])

            # ---- v tile: token-major projection + ones column ----------------
            def emit_v(tt):
                pv = p_wk.tile([128, TB], F32, tag="w", name="pv")
                tb = slice(tt * 128, (tt + 1) * 128)
                for ct in range(KT):
                    nc.tensor.matmul(
                        pv[:, 0:GF],
                        xsl(ct, tb),
                        wsl(wv_t, ct),
                        start=(ct == 0),
                        stop=(ct == KT - 1),
                    )
                v_t = p_v.tile([128, GH * (D + 1)], BF16, tag="v", name="v_t")
                vsrc = pv[:, 0:GF].rearrange("p (h d) -> p h d", h=GH)
                vdst = v_t[:].rearrange("p (h d) -> p h d", h=GH, d=D + 1)
                nc.vector.tensor_copy(vdst[:, :, 0:D], vsrc)
                nc.vector.memset(vdst[:, :, D : D + 1], 1.0)
                v_sb.append(v_t)

            # ---- attention ---------------------------------------------------
            pt_tiles = {}

            def emit_S_exp(hp, j, tt):
                """scores pair matmul + exp (+ diagonal mask on DVE)."""
                jb0 = j * TB
                r = tt - 4 * j  # >=0 on diagonal tiles
                c0 = 128 * r if r >= 0 else 0
                psp = p_mm2.tile([128, 2 * TB], F32, tag="mm2", name="psp")
                for hl in range(2):
                    hofs = hl * 64
                    nc.tensor.matmul(
                        psp[:, hl * TB + c0 : (hl + 1) * TB],
                        kh_sb[hp][hofs : hofs + 64, tt * 128 : (tt + 1) * 128],
                        qh_sb[hp][hofs : hofs + 64, jb0 + c0 : jb0 + TB],
                        start=True,
                        stop=True,
                        tile_position=(hofs, 0),
                    )
                pt = p_pt.tile([128, 2 * TB], BF16, tag="pt", name="pt")
                pt3 = pt[:].rearrange("p (h c) -> p h c", h=2)
                ps3 = psp[:].rearrange("p (h c) -> p h c", h=2)
                nc.scalar.activation(
                    pt3[:, :, c0:TB], ps3[:, :, c0:TB], AF.Exp, scale=ISD
                )
                if r >= 0:
                    m3 = mask2_sb.rearrange("p (h c) -> p h c", h=2)
                    nc.vector.tensor_mul(
                        pt3[:, :, c0 : c0 + 128], pt3[:, :, c0 : c0 + 128], m3
                    )
                pt_tiles[(hp, j)].append((tt, c0, pt))

            def emit_AV(hp, j, hl, po, tt, c0, pt):
                h = 2 * hp + hl
                nc.tensor.matmul(
                    po[0 : D + 1, c0:TB],
                    v_sb[tt][:, h * (D + 1) : (h + 1) * (D + 1)],
                    pt[:, hl * TB + c0 : (hl + 1) * TB],
                    start=(tt == 0),
                    stop=(tt == 4 * (j + 1) - 1),
                )

            def emit_div(hp, j, po0, po1):
                """merged division for both heads of the pair."""
                jb = slice(j * TB, (j + 1) * TB)
                e1 = e2b_sb[0:1, 0:64]  # ones row
                pr = p_wk.tile([128, TB], F32, tag="w", name="pr")
                for hl, po in ((0, po0), (1, po1)):
                    # 1/den = exp(-ln(den)); ACT reads the PSUM den directly
                    lnd = p_tmp.tile([1, TB], F32, tag="lnd", name="lnd", bufs=2)
                    nc.scalar.activation(lnd[:], po[D : D + 1, :], AF.Ln)
                    rc = p_tmp.tile([1, TB], BF16, tag="rc", name="rc", bufs=2)
                    nc.scalar.activation(rc[:], lnd[:], AF.Exp, scale=-1.0)
                    nc.tensor.matmul(
                        pr[hl * 64 : hl * 64 + 64, :], e1, rc[:],
                        start=True, stop=True, tile_position=(0, hl * 64),
                    )
                prb = p_tmp.tile([128, TB], BF16, tag="prb", name="prb", bufs=2)
                nc.vector.tensor_copy(prb[:], pr[:])
                nc.vector.tensor_mul(y_sb[hp][0:64, jb], po0[0:D, :], prb[0:64, :])
                nc.vector.tensor_mul(
                    y_sb[hp][64:128, jb], po1[0:D, :], prb[64:128, :]
                )

            # ---- A2A piece (hp, jp): j-pair token stripes --------------------
            cc_bufs = []

            def emit_piece(hp, jp):
                # AllGather of this rank's y for the j-pair, stripe-major so
                # receivers can select [source-slot][own-stripe] with two
                # chained dynamic (cc_rank-derived) leading-dim indices.
                # AllGather (unlike AllToAll) supports the fast Shared-output
                # HBM path.
                bin_t = p_dram.tile([4, 2, 128, nst], BF16, tag="bin",
                                    name=f"bi{hp}{jp}")
                bout_t = p_dram.tile([8, 4, 2, 128, nst], BF16, tag="bout",
                                     name=f"bo{hp}{jp}", addr_space="Shared")
                cc_bufs.append((bin_t, bout_t))
                for g in range(4):
                    for u in range(2):
                        j = 2 * jp + u
                        nc.gpsimd.dma_start(
                            bin_t[g, u],
                            y_sb[hp][:, j * TB + g * nst : j * TB + (g + 1) * nst],
                        )
                nc.gpsimd.collective_compute(
                    "AllGather",
                    mybir.AluOpType.bypass,
                    ins=[bin_t.opt()],
                    outs=[bout_t.opt()],
                    replica_groups=[[0, 1, 2, 3, 4, 5, 6, 7]],
                )
                # dynamic select: same-batch source slot, own token stripe
                for gi in range(4):
                    slot = bout_t[rb4s + gi]  # [4, 2, 128, nst]
                    stripe = slot[rg4s]  # [2, 128, nst]
                    for u in range(2):
                        nc.sync.dma_start(
                            yg_sb[2 * gi + hp][
                                :, (2 * jp + u) * nst : (2 * jp + u + 1) * nst
                            ],
                            stripe[u],
                        )

            # ---- o_proj wave over 4 slots of one hp --------------------------
            def emit_wave(hp, co, cs=slice(0, None)):
                cl = tsl if cs.stop is None else cs.stop - cs.start
                pout = p_wk.tile([128, TB], F32, tag="w", name="pout")
                for gi in range(4):
                    m = 2 * gi + hp
                    nc.tensor.matmul(
                        pout[:, 0:cl],
                        wo_t[:, m * C + co * 128 : m * C + (co + 1) * 128],
                        yg_sb[m][:, cs],
                        start=(gi == 0),
                        stop=(gi == 3),
                    )
                if hp == 0:
                    nc.vector.tensor_copy(oacc_sb[co][:, cs], pout[:, 0:cl])
                else:
                    ofin = p_tmp.tile([128, TB], F32, tag="ofin", name="ofin", bufs=2)
                    nc.vector.tensor_add(
                        ofin[:, 0:cl], pout[:, 0:cl], oacc_sb[co][:, cs]
                    )
                    nc.sync.dma_start(out[co * 128 : (co + 1) * 128, cs], ofin[:, 0:cl])

            # =================== emission schedule ===========================
            # Phase A: proj mt=0 (+ all v tiles)
            for j in range(ntb):
                emit_proj("q", 0, j)
                emit_v(4 * j + 0)
                emit_v(4 * j + 1)
                emit_proj("k", 0, j)
                emit_v(4 * j + 2)
                emit_v(4 * j + 3)

            AV_LAG = 4

            def run_attn(hp, fillers, start_after=0, tail_burst=None):
                fill_i = [0]
                total_steps = sum(4 * (j + 1) for j in range(ntb))
                per_step = len(fillers) / max(total_steps - start_after, 1)
                credit = [0.0]
                step_n = [0]
                tail = list(tail_burst or [])

                def step_fill():
                    step_n[0] += 1
                    if step_n[0] <= start_after:
                        return
                    credit[0] += per_step
                    while fill_i[0] < len(fillers) and credit[0] >= 1.0:
                        fillers[fill_i[0]]()
                        fill_i[0] += 1
                        credit[0] -= 1.0

                prev_burst = []
                for j in range(ntb):
                    pt_tiles[(hp, j)] = []
                    po0 = p_po.tile([D + 1, TB], F32, tag="po", name="po0")
                    po1 = p_po.tile([D + 1, TB], F32, tag="po", name="po1")
                    n_tt = 4 * (j + 1)
                    pend = []
                    burst = list(prev_burst)
                    for tt in range(n_tt):
                        # weave deferred work BEFORE the pt alloc in emit_S_exp
                        for _ in range(3):
                            if tail:
                                tail.pop(0)()
                        for _ in range(3):
                            if burst:
                                burst.pop(0)()
                        emit_S_exp(hp, j, tt)
                        pend.append(pt_tiles[(hp, j)][-1])
                        if len(pend) > AV_LAG:
                            ttx, c0x, ptx = pend.pop(0)
                            emit_AV(hp, j, 0, po0, ttx, c0x, ptx)
                        step_fill()
                    for item in burst:
                        item()
                    for ttx, c0x, ptx in pend:
                        emit_AV(hp, j, 0, po0, ttx, c0x, ptx)

                    if hp == 1 and j == ntb - 1:
                        # last block: run the second head's AV pass inline so
                        # the final exchange piece fires as early as possible
                        for ttx, c0x, ptx in pt_tiles[(hp, j)]:
                            emit_AV(hp, j, 1, po1, ttx, c0x, ptx)
                        emit_div(hp, j, po0, po1)
                        emit_piece(hp, j // 2)
                        prev_burst = []
                        continue

                    def make_burst(hp=hp, j=j, po0=po0, po1=po1):
                        items = []
                        for ttx, c0x, ptx in pt_tiles[(hp, j)]:
                            items.append(
                                lambda ttx=ttx, c0x=c0x, ptx=ptx: emit_AV(
                                    hp, j, 1, po1, ttx, c0x, ptx
                                )
                            )
                        items.append(lambda: emit_div(hp, j, po0, po1))
                        if j % 2 == 1:
                            items.append(lambda: emit_piece(hp, j // 2))
                        return items

                    prev_burst = make_burst()
                while fill_i[0] < len(fillers):
                    fillers[fill_i[0]]()
                    fill_i[0] += 1
                for item in tail:
                    item()
                return prev_burst

            projB = []
            for j in range(ntb):
                projB.append(lambda j=j: emit_proj("q", 1, j))
                projB.append(lambda j=j: emit_proj("k", 1, j))
            burst0 = run_attn(0, projB)

            # wave0 (hp=0) full width + first halves of wave1 as fillers
            waveC = [lambda co=co: emit_wave(0, co) for co in range(KT)]
            waveC += [
                lambda co=co: emit_wave(1, co, slice(0, tsl // 2)) for co in range(KT)
            ]
            burst1 = run_attn(1, waveC, start_after=27, tail_burst=burst0)

            # Phase D: drain last burst, final half o_proj wave + output
            for item in burst1:
                item()
            for co in range(KT):
                emit_wave(1, co, slice(tsl // 2, tsl))

    nc.compile()
    return nc


# ---------------------------------------------------------------------------
# host side
# ---------------------------------------------------------------------------


def _rope_tables(t, w):
    """[128, t] cos/sin tables with norm weight folded in (cos carries the
    output dim's weight, sin the partner dim's) and rope sign in sin."""
    inv_freq = 1.0 / (ROPE_BASE ** (np.arange(0, D, 2, dtype=np.float64) / D))
    ang = np.arange(t, dtype=np.float64)[:, None] * inv_freq[None, :]  # [t, 32]
    cos = np.cos(ang).astype(np.float32)
    sin = np.sin(ang).astype(np.float32)
    cosf = np.empty((128, t), np.float32)
    sinf = np.empty((128, t), np.float32)
    for r in range(128):
        d = r % 64
        f = d if d < 32 else d - 32
        p = d + 32 if d < 32 else d - 32
        cosf[r] = cos[:, f] * w[d]
        sinf[r] = (-sin[:, f] if d < 32 else sin[:, f]) * w[p]
    return cosf, sinf


def _consts():
    pswap = np.zeros((128, 128), np.float32)
    for j in range(128):
        d = j % 64
        i = (j - 32) if d >= 32 else (j + 32)
        pswap[i, j] = 1.0
    blk2 = np.zeros((128, 2), np.float32)
    blk2[0:64, 0] = 1.0
    blk2[64:128, 1] = 1.0
    e2b = np.zeros((2, 128), np.float32)
    e2b[0, 0:64] = 1.0
    e2b[1, 64:128] = 1.0
    mask2 = np.zeros((128, 256), np.float32)
    for p in range(128):
        mask2[p, p:128] = 1.0
        mask2[p, 128 + p : 256] = 1.0
    tabp = np.zeros((128, 516), np.float32)
    tabp[:, 0:128] = pswap
    tabp[:, 128:130] = blk2
    tabp[:, 130:386] = mask2
    tabp[0:2, 388:516] = e2b
    return tabp


def _bf(x):
    return np.ascontiguousarray(x).astype(ml_dtypes.bfloat16)


def _pack_ct(xT):
    """[C, n] -> [128, KT*n]: concat the 8 contraction tiles on columns."""
    n = xT.shape[1]
    return np.concatenate([xT[ct * 128 : (ct + 1) * 128, :] for ct in range(KT)], 1)


def make_in_maps(x, Wq, Wk, Wv, Wo, qn_w, kn_w, t=T):
    tabp = _consts()
    cosq, sinq = _rope_tables(t, qn_w)
    cosk, sink = _rope_tables(t, kn_w)
    cstab = np.concatenate([cosq, sinq, cosk, sink], 1)
    wot = np.ascontiguousarray(Wo.T)  # [c_in, c_out]
    wo_core = np.zeros((C, C), np.float32)
    for gi in range(4):
        for hp in range(2):
            u = 128 * (2 * gi + hp)
            f0 = GF * gi + 128 * hp
            wo_core[u : u + 128, :] = wot[f0 : f0 + 128, :]
    common = {
        "cstab": _bf(cstab),
        "tabp": _bf(tabp),
        "wop": _bf(_pack_ct(wo_core)),
    }
    in_maps = []
    for c in range(N_CORES):
        b, g = c // 4, c % 4
        fs = slice(GF * g, GF * (g + 1))
        xT = x[b, :t, :].T  # [C, t]
        in_maps.append(
            dict(
                common,
                xlo=_bf(_pack_ct(xT[:, 0 : t // 2])),
                xhi=_bf(_pack_ct(xT[:, t // 2 : t])),
                wqp=_bf(_pack_ct(Wq[fs, :].T)),
                wkp=_bf(_pack_ct(Wk[fs, :].T)),
                wvp=_bf(_pack_ct(Wv[fs, :].T)),
            )
        )
    return in_maps


def assemble(results, t=T):
    ntb = t // TB
    nst = (t // 4) // ntb
    out = np.empty((B, t, C), np.float32)
    for c in range(N_CORES):
        b, g = c // 4, c % 4
        r = results[c]["out"]  # [C, tsl], cols = ntb stripes of width nst
        for j in range(ntb):
            tok0 = j * TB + g * nst
            out[b, tok0 : tok0 + nst, :] = r[:, j * nst : (j + 1) * nst].T
    return out


# -- cached PJRT runner (compile once, reuse across kernel() calls) ---------

_RUNNER = {}


def _get_runner(t=T):
    if t in _RUNNER:
        return _RUNNER[t]
    import jax
    from jax.sharding import Mesh, PartitionSpec
    from jax.experimental.shard_map import shard_map
    from concourse import bass2jax

    nc = build_nc(t)
    bass2jax.install_neuronx_cc_hook()

    partition_name = nc.partition_id_tensor.name if nc.partition_id_tensor else None
    in_names = []
    out_names = []
    out_avals = []
    zero_outs = []
    for alloc in nc.m.functions[0].allocations:
        if not isinstance(alloc, mybir.MemoryLocationSet):
            continue
        name = alloc.memorylocations[0].name
        if alloc.kind == "ExternalInput":
            if name == partition_name:
                continue
            in_names.append(name)
        elif alloc.kind == "ExternalOutput":
            shape = tuple(alloc.tensor_shape)
            dtype = mybir.dt.np(alloc.dtype)
            out_names.append(name)
            out_avals.append(jax.core.ShapedArray(shape, dtype))
            zero_outs.append(np.zeros(shape, dtype))
    n_params = len(in_names)
    all_names = in_names + out_names
    if partition_name is not None:
        all_names = all_names + [partition_name]

    def _body(*args):
        operands = list(args)
        if partition_name is not None:
            operands.append(bass2jax.partition_id_tensor())
        outs = bass2jax._bass_exec_p.bind(
            *operands,
            out_avals=tuple(out_avals),
            in_names=tuple(all_names),
            out_names=tuple(out_names),
            lowering_input_output_aliases=(),
            sim_require_finite=True,
            sim_require_nnan=True,
            nc=nc,
        )
        return tuple(outs)

    devices = jax.devices()[:N_CORES]
    mesh = Mesh(np.asarray(devices), ("core",))
    fn = jax.jit(
        shard_map(
            _body,
            mesh=mesh,
            in_specs=(PartitionSpec("core"),) * (n_params + len(out_names)),
            out_specs=(PartitionSpec("core"),) * len(out_names),
            check_rep=False,
        ),
        keep_unused=True,
    )
    runner = {
        "fn": fn,
        "body": _body,
        "in_names": in_names,
        "out_names": out_names,
        "out_avals": out_avals,
        "zero_outs": zero_outs,
        "jax": jax,
    }
    _RUNNER[t] = runner
    return runner


def run_device(in_maps, t=T):
    r = _get_runner(t)
    concat_in = [
        np.concatenate([np.asarray(m[name]) for m in in_maps], axis=0)
        for name in r["in_names"]
    ]
    concat_zero = [
        np.zeros((N_CORES * z.shape[0], *z.shape[1:]), z.dtype) for z in r["zero_outs"]
    ]
    outs = r["fn"](*concat_in, *concat_zero)
    results = []
    for c in range(N_CORES):
        results.append(
            {
                name: np.asarray(outs[i]).reshape(N_CORES, *r["out_avals"][i].shape)[c]
                for i, name in enumerate(r["out_names"])
            }
        )
    return results


def kernel(x, Wq, Wk, Wv, Wo, qn_w, kn_w):
    x = np.asarray(x, np.float32)
    in_maps = make_in_maps(
        x,
        np.asarray(Wq, np.float32),
        np.asarray(Wk, np.float32),
        np.asarray(Wv, np.float32),
        np.asarray(Wo, np.float32),
        np.asarray(qn_w, np.float32),
        np.asarray(kn_w, np.float32),
    )
    results = run_device(in_maps)
    return assemble(results)


# revision 59
# speedup vs baseline: 1.0082x; 1.0082x over previous
"""Trainium2 Bass kernel for causal self-attention (RoPE + per-head RMSNorm).

Reference computation (B=2, T=2048, C=1024, H=16, D=64):
    q = rope(rmsnorm(x @ Wq.T)); k = rope(rmsnorm(x @ Wk.T)); v = x @ Wv.T
    out = softmax(causal(q k^T / sqrt(D))) v @ Wo.T

Sharding over 8 NeuronCores: core c -> batch b = c//4, head-group g = c%4
(4 heads = 256 features per group).  Feature-major ("transposed") on-chip
layout; scores computed as S^T[tk, tq]; softmax denominator via a ones
column appended to V (attn-V matmul M=65).

v2.2 structure:
  - packed host-side input layouts -> 8 big contiguous input DMAs.
  - scores for the two heads of a head-pair in one 2-bank PSUM tile
    [128,1024]; one exp activation covers both heads' tiles.
  - causal shrink: score/AV matmuls and exp skip fully-masked column
    ranges; only the diagonal 128x128 block is masked (DVE).
  - ALL reciprocals via exp(-ln(x)) on ACT (same activation table set as
    the attention exp -> zero ACT table reloads, no DVE casts).
  - engine roles: ACT = exp/ln/square only; DVE = elementwise+masks;
    GPSIMD = rope mul/add (slack-tolerant) + collectives + bounce DMA;
    PE = all matmuls; sync = loads/unbounce/output.
  - dense PE stream: proj(mt=1) woven into attention(hp=0), o_proj wave0
    woven into attention(hp=1), second-head AV passes woven into the next
    query block's score stream.
  - AllToAll split into 4 pieces (hp x query-block pairs); received slots
    selected dynamically via cc_rank (no junk slots, halved wo/o_proj).
  - last o_proj wave split into column halves so only the final quarter
    of the A2A + a small wave is exposed at the tail.
"""

import os
import sys

for _p in ("/opt/trn_rl_repo", "/root/.axon_site/_ro/trn_rl_repo"):
    if os.path.isdir(_p) and _p not in sys.path:
        sys.path.insert(0, _p)

import numpy as np
import ml_dtypes

import concourse.bass as bass
from concourse import bacc
import concourse.tile as tile
import concourse.mybir as mybir

# Pin activation-table selection to the one set that covers every ACT
# function this kernel uses (exp/ln/square/copy/identity).  The default
# greedy per-instruction selection alternates between the exp-only and
# ln-only sets, inserting a 1.3us ACT_TABLE_LOAD at every switch (60+
# loads).  Masking the other sets (positions preserved, so the emitted
# act_func_set_id still indexes act_info.json correctly) forces every
# activation onto the combined resident table -> one load total.
_COMBO_SET = "natural_log_exp_and_others"
_orig_get_tables = bacc.get_activation_tables


def _mono_tables(arch):
    t = _orig_get_tables(arch)
    if _COMBO_SET not in t:
        return t
    return {k: (v if k == _COMBO_SET else set()) for k, v in t.items()}


bacc.get_activation_tables = _mono_tables

BF16 = mybir.dt.bfloat16
F32 = mybir.dt.float32
AF = mybir.ActivationFunctionType

B, T, C, H, D = 2, 2048, 1024, 16, 64
N_CORES = 8
GH = 4  # heads per core
GF = GH * D  # features per core (256)
TB = 512  # token block (matmul N)
KT = C // 128  # 8 contraction k-tiles
EPS = float(np.finfo(np.float32).eps)
ROPE_BASE = 10000.0
ISD = 1.0 / np.sqrt(D)


def build_nc(t=T):
    ntb = t // TB  # query blocks (4)
    ntt = t // 128  # token 128-tiles (16)
    tsl = t // 4  # per-core o_proj token count (512)
    nst = tsl // ntb  # stripe width (128)
    th = t // 2

    nc = bacc.Bacc("TRN2", target_bir_lowering=False, debug=False, num_devices=N_CORES)

    # packed inputs (one contiguous DMA each)
    xlo = nc.dram_tensor("xlo", [128, KT * th], BF16, kind="ExternalInput")
    xhi = nc.dram_tensor("xhi", [128, KT * th], BF16, kind="ExternalInput")
    wqp = nc.dram_tensor("wqp", [128, KT * GF], BF16, kind="ExternalInput")
    wkp = nc.dram_tensor("wkp", [128, KT * GF], BF16, kind="ExternalInput")
    wvp = nc.dram_tensor("wvp", [128, KT * GF], BF16, kind="ExternalInput")
    wop = nc.dram_tensor("wop", [128, KT * C], BF16, kind="ExternalInput")
    cstab = nc.dram_tensor("cstab", [128, 4 * t], BF16, kind="ExternalInput")
    tabp = nc.dram_tensor("tabp", [128, 516], BF16, kind="ExternalInput")
    out = nc.dram_tensor("out", [C, tsl], F32, kind="ExternalOutput")

    with tile.TileContext(nc) as tc:
        with (
            nc.allow_low_precision(reason="bf16 compute by design"),
            tc.tile_pool(name="p_x", bufs=2) as p_x,
            tc.tile_pool(name="p_w", bufs=3) as p_w,
            tc.tile_pool(name="p_tab", bufs=1) as p_tab,
            tc.tile_pool(name="p_qk", bufs=2) as p_qk,
            tc.tile_pool(name="p_v", bufs=ntt) as p_v,
            tc.tile_pool(name="p_y", bufs=1) as p_y,
            tc.tile_pool(name="p_yg", bufs=KT) as p_yg,
            tc.tile_pool(name="p_pt", bufs=14) as p_pt,
            tc.tile_pool(name="p_tmp", bufs=3) as p_tmp,
            tc.tile_pool(name="p_oacc", bufs=KT) as p_oacc,
            tc.tile_pool(name="p_mm2", bufs=2, space="PSUM") as p_mm2,
            tc.tile_pool(name="p_po", bufs=2, space="PSUM") as p_po,
            tc.tile_pool(name="p_wk", bufs=2, space="PSUM") as p_wk,
            tc.tile_pool(name="p_dram", bufs=2, space="DRAM") as p_dram,
        ):
            # ---- input loads: big DMAs on the sync+scalar queues only.
            # The gpsimd queue is unusable for ~45us at kernel start (the
            # collectives-init op occupies it), so nothing startup-critical
            # goes there.
            tab_t = p_tab.tile([128, 516], BF16, tag="tab", name="tab_t")
            nc.sync.dma_start(tab_t[:], tabp[:])
            wq_t = p_w.tile([128, KT * GF], BF16, tag="wq", name="wq_t", bufs=1)
            nc.scalar.dma_start(wq_t[:], wqp[:])
            xlo_t = p_x.tile([128, KT * th], BF16, tag="xl", name="xlo_t", bufs=1)
            nc.sync.dma_start(xlo_t[:, 0 : 4 * th], xlo[:, 0 : 4 * th])
            nc.scalar.dma_start(xlo_t[:, 4 * th : 8 * th], xlo[:, 4 * th : 8 * th])
            wv_t = p_w.tile([128, KT * GF], BF16, tag="wv", name="wv_t", bufs=1)
            nc.scalar.dma_start(wv_t[:], wvp[:])
            wk_t = p_w.tile([128, KT * GF], BF16, tag="wk", name="wk_t", bufs=1)
            nc.scalar.dma_start(wk_t[:], wkp[:])
            xhi_t = p_x.tile([128, KT * th], BF16, tag="xh", name="xhi_t", bufs=1)
            nc.sync.dma_start(xhi_t[:, 0 : 4 * th], xhi[:, 0 : 4 * th])
            nc.sync.dma_start(xhi_t[:, 4 * th : 8 * th], xhi[:, 4 * th : 8 * th])
            cs_t = p_tab.tile([128, 4 * t], BF16, tag="cs", name="cs_t")
            nc.scalar.dma_start(cs_t[:, 0 : 2 * t], cstab[:, 0 : 2 * t])
            nc.sync.dma_start(cs_t[:, 2 * t : 4 * t], cstab[:, 2 * t : 4 * t])
            wo_t = p_w.tile([128, KT * C], BF16, tag="wo", name="wo_t", bufs=1)
            nc.sync.dma_start(wo_t[:], wop[:])

            def xsl(ct, jb):  # x^T tile ct, token slice jb (global cols)
                lo, hi = jb.start, jb.stop
                if hi <= th:
                    return xlo_t[:, ct * th + lo : ct * th + hi]
                return xhi_t[:, ct * th + lo - th : ct * th + hi - th]

            def wsl(w, ct, mt=None):
                base = w[:, ct * GF : (ct + 1) * GF]
                if mt is None:
                    return base
                return w[:, ct * GF + mt * 128 : ct * GF + (mt + 1) * 128]

            pswap_sb = tab_t[:, 0:128]
            blk2_sb = tab_t[:, 128:130]
            mask2_sb = tab_t[:, 130:386]
            e2b_sb = tab_t[0:2, 388:516]
            cosq_sb = cs_t[:, 0 * t : 1 * t]
            sinq_sb = cs_t[:, 1 * t : 2 * t]
            cosk_sb = cs_t[:, 2 * t : 3 * t]
            sink_sb = cs_t[:, 3 * t : 4 * t]
            eps_sb = p_tab.tile([128, 1], F32, tag="eps")
            nc.vector.memset(eps_sb[:], EPS)

            # register with 4*(rank//4) for dynamic slot selection from A2A
            rb_reg = nc.sync.alloc_register()
            nc.sync.cc_rank_ld(rb_reg, replica_groups=[[0, 1, 2, 3, 4, 5, 6, 7]])
            rg_reg = nc.sync.alloc_register()
            nc.sync.reg_mod(rg_reg, rb_reg, 4)
            nc.sync.reg_div(rb_reg, rb_reg, 4)
            nc.sync.reg_mul(rb_reg, rb_reg, 4)
            rb4s = nc.sync.snap(rb_reg, min_val=0, max_val=4)
            rg4s = nc.sync.snap(rg_reg, min_val=0, max_val=3)

            qh_sb = [p_qk.tile([128, t], BF16, tag="qk0", name="qh0"),
                     p_qk.tile([128, t], BF16, tag="qk1", name="qh1")]
            kh_sb = [p_qk.tile([128, t], BF16, tag="qk0", name="kh0"),
                     p_qk.tile([128, t], BF16, tag="qk1", name="kh1")]
            v_sb = []
            y_sb = [p_y.tile([128, t], BF16, tag="y0", name="y0"),
                    p_y.tile([128, t], BF16, tag="y1", name="y1")]
            yg_sb = [None] * KT  # index m = 2*gi + hp
            for i in range(KT):
                yg_sb[i] = p_yg.tile([128, tsl], BF16, tag="yg", name=f"yg{i}")
            oacc_sb = []
            for co in range(KT):
                o_t = p_oacc.tile([128, tsl], BF16, tag="oacc", name=f"oac{co}")
                oacc_sb.append(o_t)

            # ---- proj block: 8 acc MMs + rmsnorm/rope chain ------------------
            def emit_proj(which, mt, j):
                jb = slice(j * TB, (j + 1) * TB)
                w_t = wq_t if which == "q" else wk_t
                cos_sb = cosq_sb if which == "q" else cosk_sb
                sin_sb = sinq_sb if which == "q" else sink_sb
                dst = (qh_sb if which == "q" else kh_sb)[mt]
                acc = p_wk.tile([128, TB], F32, tag="w", name="acc")
                for ct in range(KT):
                    nc.tensor.matmul(
                        acc[:],
                        wsl(w_t, ct, mt),
                        xsl(ct, jb),
                        start=(ct == 0),
                        stop=(ct == KT - 1),
                    )
                # rmsnorm deferred past rope: inv is constant within a head's
                # 64 rows, so it commutes with the half-swap rotation.
                pqb = p_tmp.tile([128, TB], BF16, tag="qn", name="pqb")
                nc.vector.tensor_copy(pqb[:], acc[:])
                sq = p_tmp.tile([128, TB], BF16, tag="sq", name="sq", bufs=2)
                nc.vector.tensor_mul(sq[:], pqb[:], pqb[:])
                # pw holds the [2,TB] sum-of-squares first, then (after ln
                # consumed it) the [128,TB] broadcast inv: one PSUM bank.
                pw = p_wk.tile([128, TB], F32, tag="w", name="pw")
                nc.tensor.matmul(pw[0:2, :], blk2_sb, sq[:], start=True, stop=True)
                # inv = 1/sqrt(ms+eps) = exp(-0.5*ln(ms+eps)): stays in the
                # exp/ln ACT table set (no table reloads).
                lnv = p_tmp.tile([2, TB], F32, tag="lnv", name="lnv", bufs=2)
                nc.scalar.activation(
                    lnv[:], pw[0:2, :], AF.Ln, scale=1.0 / D, bias=eps_sb[0:2, :]
                )
                invb = p_tmp.tile([2, TB], BF16, tag="invb", name="invb", bufs=2)
                nc.scalar.activation(invb[:], lnv[:], AF.Exp, scale=-0.5)
                nc.tensor.matmul(pw[:], e2b_sb, invb[:], start=True, stop=True)
                pqs = p_wk.tile([128, TB], F32, tag="w", name="pqs")
                nc.tensor.matmul(pqs[:], pswap_sb, pqb[:], start=True, stop=True)
                t1 = p_tmp.tile([128, TB], BF16, tag="t1", name="t1")
                nc.gpsimd.tensor_mul(t1[:], pqb[:], cos_sb[:, jb])
                t2 = p_tmp.tile([128, TB], BF16, tag="t2", name="t2")
                nc.vector.tensor_mul(t2[:], pqs[:], sin_sb[:, jb])
                rsum = p_tmp.tile([128, TB], BF16, tag="rs", name="rsum")
                nc.gpsimd.tensor_add(rsum[:], t1[:], t2[:])
                nc.vector.tensor_mul(dst[:, jb], rsum[:], pw[:])

            # ---- v tile: token-major projection + ones column ----------------
            def emit_v(tt):
                pv = p_wk.tile([128, TB], F32, tag="w", name="pv")
                tb = slice(tt * 128, (tt + 1) * 128)
                for ct in range(KT):
                    nc.tensor.matmul(
                        pv[:, 0:GF],
                        xsl(ct, tb),
                        wsl(wv_t, ct),
                        start=(ct == 0),
                        stop=(ct == KT - 1),
                    )
                v_t = p_v.tile([128, GH * (D + 1)], BF16, tag="v", name="v_t")
                vsrc = pv[:, 0:GF].rearrange("p (h d) -> p h d", h=GH)
                vdst = v_t[:].rearrange("p (h d) -> p h d", h=GH, d=D + 1)
                nc.vector.tensor_copy(vdst[:, :, 0:D], vsrc)
                nc.vector.memset(vdst[:, :, D : D + 1], 1.0)
                v_sb.append(v_t)

            # ---- attention ---------------------------------------------------
            pt_tiles = {}

            def emit_S_exp(hp, j, tt):
                """scores pair matmul + exp (+ diagonal mask on DVE)."""
                jb0 = j * TB
                r = tt - 4 * j  # >=0 on diagonal tiles
                c0 = 128 * r if r >= 0 else 0
                psp = p_mm2.tile([128, 2 * TB], F32, tag="mm2", name="psp")
                for hl in range(2):
                    hofs = hl * 64
                    nc.tensor.matmul(
                        psp[:, hl * TB + c0 : (hl + 1) * TB],
                        kh_sb[hp][hofs : hofs + 64, tt * 128 : (tt + 1) * 128],
                        qh_sb[hp][hofs : hofs + 64, jb0 + c0 : jb0 + TB],
                        start=True,
                        stop=True,
                        tile_position=(hofs, 0),
                    )
                pt = p_pt.tile([128, 2 * TB], BF16, tag="pt", name="pt")
                pt3 = pt[:].rearrange("p (h c) -> p h c", h=2)
                ps3 = psp[:].rearrange("p (h c) -> p h c", h=2)
                nc.scalar.activation(
                    pt3[:, :, c0:TB], ps3[:, :, c0:TB], AF.Exp, scale=ISD
                )
                if r >= 0:
                    m3 = mask2_sb.rearrange("p (h c) -> p h c", h=2)
                    nc.vector.tensor_mul(
                        pt3[:, :, c0 : c0 + 128], pt3[:, :, c0 : c0 + 128], m3
                    )
                pt_tiles[(hp, j)].append((tt, c0, pt))

            def emit_AV(hp, j, hl, po, tt, c0, pt):
                h = 2 * hp + hl
                nc.tensor.matmul(
                    po[0 : D + 1, c0:TB],
                    v_sb[tt][:, h * (D + 1) : (h + 1) * (D + 1)],
                    pt[:, hl * TB + c0 : (hl + 1) * TB],
                    start=(tt == 0),
                    stop=(tt == 4 * (j + 1) - 1),
                )

            def emit_div(hp, j, po0, po1):
                """merged division for both heads of the pair."""
                jb = slice(j * TB, (j + 1) * TB)
                e1 = e2b_sb[0:1, 0:64]  # ones row
                pr = p_wk.tile([128, TB], F32, tag="w", name="pr")
                for hl, po in ((0, po0), (1, po1)):
                    # 1/den = exp(-ln(den)); ACT reads the PSUM den directly
                    lnd = p_tmp.tile([1, TB], F32, tag="lnd", name="lnd", bufs=2)
                    nc.scalar.activation(lnd[:], po[D : D + 1, :], AF.Ln)
                    rc = p_tmp.tile([1, TB], BF16, tag="rc", name="rc", bufs=2)
                    nc.scalar.activation(rc[:], lnd[:], AF.Exp, scale=-1.0)
                    nc.tensor.matmul(
                        pr[hl * 64 : hl * 64 + 64, :], e1, rc[:],
                        start=True, stop=True, tile_position=(0, hl * 64),
                    )
                prb = p_tmp.tile([128, TB], BF16, tag="prb", name="prb", bufs=2)
                nc.vector.tensor_copy(prb[:], pr[:])
                nc.vector.tensor_mul(y_sb[hp][0:64, jb], po0[0:D, :], prb[0:64, :])
                nc.vector.tensor_mul(
                    y_sb[hp][64:128, jb], po1[0:D, :], prb[64:128, :]
                )

            # ---- A2A piece (hp, jp): j-pair token stripes --------------------
            cc_bufs = []

            def emit_piece(hp, jp):
                # AllGather of this rank's y for the j-pair, stripe-major so
                # receivers can select [source-slot][own-stripe] with two
                # chained dynamic (cc_rank-derived) leading-dim indices.
                # AllGather (unlike AllToAll) supports the fast Shared-output
                # HBM path.
                bin_t = p_dram.tile([4, 2, 128, nst], BF16, tag="bin",
                                    name=f"bi{hp}{jp}")
                bout_t = p_dram.tile([8, 4, 2, 128, nst], BF16, tag="bout",
                                     name=f"bo{hp}{jp}", addr_space="Shared")
                cc_bufs.append((bin_t, bout_t))
                for g in range(4):
                    for u in range(2):
                        j = 2 * jp + u
                        nc.gpsimd.dma_start(
                            bin_t[g, u],
                            y_sb[hp][:, j * TB + g * nst : j * TB + (g + 1) * nst],
                        )
                nc.gpsimd.collective_compute(
                    "AllGather",
                    mybir.AluOpType.bypass,
                    ins=[bin_t.opt()],
                    outs=[bout_t.opt()],
                    replica_groups=[[0, 1, 2, 3, 4, 5, 6, 7]],
                )
                # dynamic select: same-batch source slot, own token stripe
                for gi in range(4):
                    slot = bout_t[rb4s + gi]  # [4, 2, 128, nst]
                    stripe = slot[rg4s]  # [2, 128, nst]
                    for u in range(2):
                        nc.sync.dma_start(
                            yg_sb[2 * gi + hp][
                                :, (2 * jp + u) * nst : (2 * jp + u + 1) * nst
                            ],
                            stripe[u],
                        )

            # ---- o_proj wave over 4 slots of one hp --------------------------
            def emit_wave(hp, co, cs=slice(0, None)):
                cl = tsl if cs.stop is None else cs.stop - cs.start
                pout = p_wk.tile([128, TB], F32, tag="w", name="pout")
                for gi in range(4):
                    m = 2 * gi + hp
                    nc.tensor.matmul(
                        pout[:, 0:cl],
                        wo_t[:, m * C + co * 128 : m * C + (co + 1) * 128],
                        yg_sb[m][:, cs],
                        start=(gi == 0),
                        stop=(gi == 3),
                    )
                if hp == 0:
                    nc.vector.tensor_copy(oacc_sb[co][:, cs], pout[:, 0:cl])
                else:
                    ofin = p_tmp.tile([128, TB], F32, tag="ofin", name="ofin", bufs=2)
                    nc.vector.tensor_add(
                        ofin[:, 0:cl], pout[:, 0:cl], oacc_sb[co][:, cs]
                    )
                    nc.sync.dma_start(out[co * 128 : (co + 1) * 128, cs], ofin[:, 0:cl])

            # =================== emission schedule ===========================
            # Phase A: proj mt=0 (+ all v tiles)
            for j in range(ntb):
                emit_proj("q", 0, j)
                emit_v(4 * j + 0)
                emit_v(4 * j + 1)
                emit_proj("k", 0, j)
                emit_v(4 * j + 2)
                emit_v(4 * j + 3)

            AV_LAG = 4

            def run_attn(hp, fillers, start_after=0, tail_burst=None):
                fill_i = [0]
                total_steps = sum(4 * (j + 1) for j in range(ntb))
                per_step = len(fillers) / max(total_steps - start_after, 1)
                credit = [0.0]
                step_n = [0]
                tail = list(tail_burst or [])

                def step_fill():
                    step_n[0] += 1
                    if step_n[0] <= start_after:
                        return
                    credit[0] += per_step
                    while fill_i[0] < len(fillers) and credit[0] >= 1.0:
                        fillers[fill_i[0]]()
                        fill_i[0] += 1
                        credit[0] -= 1.0

                prev_burst = []
                for j in range(ntb):
                    pt_tiles[(hp, j)] = []
                    po0 = p_po.tile([D + 1, TB], F32, tag="po", name="po0")
                    po1 = p_po.tile([D + 1, TB], F32, tag="po", name="po1")
                    n_tt = 4 * (j + 1)
                    pend = []
                    burst = list(prev_burst)
                    for tt in range(n_tt):
                        # weave deferred work BEFORE the pt alloc in emit_S_exp
                        for _ in range(3):
                            if tail:
                                tail.pop(0)()
                        for _ in range(3):
                            if burst:
                                burst.pop(0)()
                        emit_S_exp(hp, j, tt)
                        pend.append(pt_tiles[(hp, j)][-1])
                        if len(pend) > AV_LAG:
                            ttx, c0x, ptx = pend.pop(0)
                            emit_AV(hp, j, 0, po0, ttx, c0x, ptx)
                        step_fill()
                    for item in burst:
                        item()
                    for ttx, c0x, ptx in pend:
                        emit_AV(hp, j, 0, po0, ttx, c0x, ptx)

                    if hp == 1 and j == ntb - 1:
                        # last block: run the second head's AV pass inline so
                        # the final exchange piece fires as early as possible
                        for ttx, c0x, ptx in pt_tiles[(hp, j)]:
                            emit_AV(hp, j, 1, po1, ttx, c0x, ptx)
                        emit_div(hp, j, po0, po1)
                        emit_piece(hp, j // 2)
                        prev_burst = []
                        continue

                    def make_burst(hp=hp, j=j, po0=po0, po1=po1):
                        items = []
                        for ttx, c0x, ptx in pt_tiles[(hp, j)]:
                            items.append(
                                lambda ttx=ttx, c0x=c0x, ptx=ptx: emit_AV(
                                    hp, j, 1, po1, ttx, c0x, ptx
                                )
                            )
                        items.append(lambda: emit_div(hp, j, po0, po1))
                        if j % 2 == 1:
                            items.append(lambda: emit_piece(hp, j // 2))
                        return items

                    prev_burst = make_burst()
                while fill_i[0] < len(fillers):
                    fillers[fill_i[0]]()
                    fill_i[0] += 1
                for item in tail:
                    item()
                return prev_burst

            projB = []
            for j in range(ntb):
                projB.append(lambda j=j: emit_proj("q", 1, j))
                projB.append(lambda j=j: emit_proj("k", 1, j))
            burst0 = run_attn(0, projB)

            # wave0 (hp=0) full width + first halves of wave1 as fillers
            waveC = [lambda co=co: emit_wave(0, co) for co in range(KT)]
            waveC += [
                lambda co=co: emit_wave(1, co, slice(0, tsl // 2)) for co in range(KT)
            ]
            burst1 = run_attn(1, waveC, start_after=24, tail_burst=burst0)

            # Phase D: drain last burst, final half o_proj wave + output
            for item in burst1:
                item()
            for co in range(KT):
                emit_wave(1, co, slice(tsl // 2, tsl))

    nc.compile()
    return nc


# ---------------------------------------------------------------------------
# host side
# ---------------------------------------------------------------------------


def _rope_tables(t, w):
    """[128, t] cos/sin tables with norm weight folded in (cos carries the
    output dim's weight, sin the partner dim's) and rope sign in sin."""
    inv_freq = 1.0 / (ROPE_BASE ** (np.arange(0, D, 2, dtype=np.float64) / D))
    ang = np.arange(t, dtype=np.float64)[:, None] * inv_freq[None, :]  # [t, 32]
    cos = np.cos(ang).astype(np.float32)
    sin = np.sin(ang).astype(np.float32)
    cosf = np.empty((128, t), np.float32)
    sinf = np.empty((128, t), np.float32)
    for r in range(128):
        d = r % 64
        f = d if d < 32 else d - 32
        p = d + 32 if d < 32 else d - 32
        cosf[r] = cos[:, f] * w[d]
        sinf[r] = (-sin[:, f] if d < 32 else sin[:, f]) * w[p]
    return cosf, sinf


def _consts():
    pswap = np.zeros((128, 128), np.float32)
    for j in range(128):
        d = j % 64
        i = (j - 32) if d >= 32 else (j + 32)
        pswap[i, j] = 1.0
    blk2 = np.zeros((128, 2), np.float32)
    blk2[0:64, 0] = 1.0
    blk2[64:128, 1] = 1.0
    e2b = np.zeros((2, 128), np.float32)
    e2b[0, 0:64] = 1.0
    e2b[1, 64:128] = 1.0
    mask2 = np.zeros((128, 256), np.float32)
    for p in range(128):
        mask2[p, p:128] = 1.0
        mask2[p, 128 + p : 256] = 1.0
    tabp = np.zeros((128, 516), np.float32)
    tabp[:, 0:128] = pswap
    tabp[:, 128:130] = blk2
    tabp[:, 130:386] = mask2
    tabp[0:2, 388:516] = e2b
    return tabp


def _bf(x):
    return np.ascontiguousarray(x).astype(ml_dtypes.bfloat16)


def _pack_ct(xT):
    """[C, n] -> [128, KT*n]: concat the 8 contraction tiles on columns."""
    n = xT.shape[1]
    return np.concatenate([xT[ct * 128 : (ct + 1) * 128, :] for ct in range(KT)], 1)


def make_in_maps(x, Wq, Wk, Wv, Wo, qn_w, kn_w, t=T):
    tabp = _consts()
    cosq, sinq = _rope_tables(t, qn_w)
    cosk, sink = _rope_tables(t, kn_w)
    cstab = np.concatenate([cosq, sinq, cosk, sink], 1)
    wot = np.ascontiguousarray(Wo.T)  # [c_in, c_out]
    wo_core = np.zeros((C, C), np.float32)
    for gi in range(4):
        for hp in range(2):
            u = 128 * (2 * gi + hp)
            f0 = GF * gi + 128 * hp
            wo_core[u : u + 128, :] = wot[f0 : f0 + 128, :]
    common = {
        "cstab": _bf(cstab),
        "tabp": _bf(tabp),
        "wop": _bf(_pack_ct(wo_core)),
    }
    in_maps = []
    for c in range(N_CORES):
        b, g = c // 4, c % 4
        fs = slice(GF * g, GF * (g + 1))
        xT = x[b, :t, :].T  # [C, t]
        in_maps.append(
            dict(
                common,
                xlo=_bf(_pack_ct(xT[:, 0 : t // 2])),
                xhi=_bf(_pack_ct(xT[:, t // 2 : t])),
                wqp=_bf(_pack_ct(Wq[fs, :].T)),
                wkp=_bf(_pack_ct(Wk[fs, :].T)),
                wvp=_bf(_pack_ct(Wv[fs, :].T)),
            )
        )
    return in_maps


def assemble(results, t=T):
    ntb = t // TB
    nst = (t // 4) // ntb
    out = np.empty((B, t, C), np.float32)
    for c in range(N_CORES):
        b, g = c // 4, c % 4
        r = results[c]["out"]  # [C, tsl], cols = ntb stripes of width nst
        for j in range(ntb):
            tok0 = j * TB + g * nst
            out[b, tok0 : tok0 + nst, :] = r[:, j * nst : (j + 1) * nst].T
    return out


# -- cached PJRT runner (compile once, reuse across kernel() calls) ---------

_RUNNER = {}


def _get_runner(t=T):
    if t in _RUNNER:
        return _RUNNER[t]
    import jax
    from jax.sharding import Mesh, PartitionSpec
    from jax.experimental.shard_map import shard_map
    from concourse import bass2jax

    nc = build_nc(t)
    bass2jax.install_neuronx_cc_hook()

    partition_name = nc.partition_id_tensor.name if nc.partition_id_tensor else None
    in_names = []
    out_names = []
    out_avals = []
    zero_outs = []
    for alloc in nc.m.functions[0].allocations:
        if not isinstance(alloc, mybir.MemoryLocationSet):
            continue
        name = alloc.memorylocations[0].name
        if alloc.kind == "ExternalInput":
            if name == partition_name:
                continue
            in_names.append(name)
        elif alloc.kind == "ExternalOutput":
            shape = tuple(alloc.tensor_shape)
            dtype = mybir.dt.np(alloc.dtype)
            out_names.append(name)
            out_avals.append(jax.core.ShapedArray(shape, dtype))
            zero_outs.append(np.zeros(shape, dtype))
    n_params = len(in_names)
    all_names = in_names + out_names
    if partition_name is not None:
        all_names = all_names + [partition_name]

    def _body(*args):
        operands = list(args)
        if partition_name is not None:
            operands.append(bass2jax.partition_id_tensor())
        outs = bass2jax._bass_exec_p.bind(
            *operands,
            out_avals=tuple(out_avals),
            in_names=tuple(all_names),
            out_names=tuple(out_names),
            lowering_input_output_aliases=(),
            sim_require_finite=True,
            sim_require_nnan=True,
            nc=nc,
        )
        return tuple(outs)

    devices = jax.devices()[:N_CORES]
    mesh = Mesh(np.asarray(devices), ("core",))
    fn = jax.jit(
        shard_map(
            _body,
            mesh=mesh,
            in_specs=(PartitionSpec("core"),) * (n_params + len(out_names)),
            out_specs=(PartitionSpec("core"),) * len(out_names),
            check_rep=False,
        ),
        keep_unused=True,
    )
    runner = {
        "fn": fn,
        "body": _body,
        "in_names": in_names,
        "out_names": out_names,
        "out_avals": out_avals,
        "zero_outs": zero_outs,
        "jax": jax,
    }
    _RUNNER[t] = runner
    return runner


def run_device(in_maps, t=T):
    r = _get_runner(t)
    concat_in = [
        np.concatenate([np.asarray(m[name]) for m in in_maps], axis=0)
        for name in r["in_names"]
    ]
    concat_zero = [
        np.zeros((N_CORES * z.shape[0], *z.shape[1:]), z.dtype) for z in r["zero_outs"]
    ]
    outs = r["fn"](*concat_in, *concat_zero)
    results = []
    for c in range(N_CORES):
        results.append(
            {
                name: np.asarray(outs[i]).reshape(N_CORES, *r["out_avals"][i].shape)[c]
                for i, name in enumerate(r["out_names"])
            }
        )
    return results


def kernel(x, Wq, Wk, Wv, Wo, qn_w, kn_w):
    x = np.asarray(x, np.float32)
    in_maps = make_in_maps(
        x,
        np.asarray(Wq, np.float32),
        np.asarray(Wk, np.float32),
        np.asarray(Wv, np.float32),
        np.asarray(Wo, np.float32),
        np.asarray(qn_w, np.float32),
        np.asarray(kn_w, np.float32),
    )
    results = run_device(in_maps)
    return assemble(results)
